# revision 6
# baseline (speedup 1.0000x reference)
"""Trainium2 Bass kernel for the CoAtt_P problem.

Computes, for q:[B,Lq,D], v:[B,Lv,D], w:[D,D]:
    qw   = q @ w                      [B,Lq,D]
    S    = qw @ v^T                   [B,Lq,Lv]   (scores; tanh deferred)
    m_v  = tanh(max_i S[:,i,:])       [B,Lv]      (tanh is monotone: tanh(max)=max(tanh))
    m_q  = tanh(max_j S[:,:,j])       [B,Lq]
    att_v = softmax(m_v) @ v          [B,D]
    att_q = softmax(m_q) @ q          [B,D]
returns (att_q, att_v).

Sharding: data-parallel over the batch dim across 8 NeuronCores (8 batches
per core); w replicated. All large matmuls run in bf16 (inputs converted on
host), fp32 PSUM accumulation; the softmax weights operate on tanh outputs
in [-1,1] so no max-subtraction is needed for stability.
"""

import sys
import types

import numpy as np
import ml_dtypes
from contextlib import ExitStack

# The NTFF profiling hook module is absent from this image's antenv package;
# shim it so run_bass_kernel_spmd(trace=True) works when test harnesses ask
# for a profile. Harmless when tracing is never requested.
if "antenv.axon_hooks" not in sys.modules:
    _m = types.ModuleType("antenv.axon_hooks")
    _m._hook = None
    _m.set_axon_ntff_profile_hook = lambda h: setattr(_m, "_hook", h)
    _m.get_axon_ntff_profile_hook = lambda: _m._hook
    sys.modules["antenv.axon_hooks"] = _m
    try:
        import antenv

        antenv.axon_hooks = _m
        from trn_agent_boot.trn_boot import _ntff_profile_via_ctypes

        _m.set_axon_ntff_profile_hook(
            _ntff_profile_via_ctypes("/opt/axon/libaxon_pjrt.so")
        )
    except Exception:
        pass

from concourse import tile, bacc, mybir
from concourse.bass import ts
from concourse.bass_utils import run_bass_kernel_spmd
from concourse.masks import make_identity

BF16 = mybir.dt.bfloat16
F32 = mybir.dt.float32
MAX = mybir.AluOpType.max
AX = mybir.AxisListType.X

B, L, D = 64, 1024, 256
NCORES = 8
BPC = B // NCORES  # batches per core
LT = L // 128      # 128-row tiles along Lq/Lv
DC = D // 128      # 128-wide chunks along D
NEG = -1.0e30

# Score tiles are copied PSUM->SBUF(bf16) on ScalarE; row-max and the running
# column max run on VectorE from the bf16 copy (2x/4x DVE modes).
# (tensor_tensor_reduce would fuse copy+rowmax but crashes this runtime.)


def _build():
    nc = bacc.Bacc(None, target_bir_lowering=False)
    q_d = nc.dram_tensor("q", [BPC, L, D], BF16, kind="ExternalInput")
    v_d = nc.dram_tensor("v", [BPC, L, D], BF16, kind="ExternalInput")
    w_d = nc.dram_tensor("w", [D, D], BF16, kind="ExternalInput")
    o_d = nc.dram_tensor("out", [2, BPC, D], F32, kind="ExternalOutput")

    with ExitStack() as ctx:
        tc = ctx.enter_context(tile.TileContext(nc))
        singles = ctx.enter_context(tc.tile_pool(name="singles", bufs=1))
        pio = ctx.enter_context(tc.tile_pool(name="pio", bufs=2))
        psb = ctx.enter_context(tc.tile_pool(name="psb", bufs=3))
        patt = ctx.enter_context(tc.tile_pool(name="patt", bufs=4))
        pbig = ctx.enter_context(tc.tile_pool(name="pbig", bufs=3, space="PSUM"))
        pacc = ctx.enter_context(tc.tile_pool(name="pacc", bufs=2, space="PSUM"))

        ident = singles.tile([128, 128], BF16)
        make_identity(nc, ident)
        # w laid out [d_in%128, d_in//128, d_out] so w_sb[:, kc, mc*128:...]
        # is the [K=128, M=128] stationary chunk of w for the qw matmul.
        w_sb = singles.tile([128, DC, D], BF16)
        nc.sync.dma_start(out=w_sb, in_=w_d.rearrange("(kc p) e -> p kc e", p=128))
        ones_col = singles.tile([128, 1], F32)
        nc.vector.memset(ones_col, 1.0)

        for b in range(BPC):
            # --- loads: native [lq%128, lq//128, d] and transposed [d%128, d//128, l]
            q_nat = pio.tile([128, LT, D], BF16, tag="q_nat")
            nc.sync.dma_start(out=q_nat, in_=q_d[b].rearrange("(t p) d -> p t d", p=128))
            v_nat = pio.tile([128, LT, D], BF16, tag="v_nat")
            nc.sync.dma_start(out=v_nat, in_=v_d[b].rearrange("(t p) d -> p t d", p=128))
            qT = pio.tile([128, DC, L], BF16, tag="qT")
            vT = pio.tile([128, DC, L], BF16, tag="vT")
            for c in range(DC):
                nc.sync.dma_start(out=qT[:, c, :], in_=q_d[b][:, ts(c, 128)], transpose=True)
                nc.sync.dma_start(out=vT[:, c, :], in_=v_d[b][:, ts(c, 128)], transpose=True)

            # --- qw^T[d_out, lq] = sum_{d_in} w[d_in, d_out] * q^T[d_in, lq]
            qwT = pio.tile([128, DC, L], BF16, tag="qwT")
            for mc in range(DC):
                ps_qw = pbig.tile([128, L], F32, tag="big")
                for n in range(2):
                    for kc in range(DC):
                        nc.tensor.matmul(
                            ps_qw[:, ts(n, 512)],
                            lhsT=w_sb[:, kc, ts(mc, 128)],
                            rhs=qT[:, kc, ts(n, 512)],
                            start=(kc == 0),
                            stop=(kc == DC - 1),
                        )
                nc.scalar.copy(out=qwT[:, mc, :], in_=ps_qw)

            # --- scores S[t] = qw^T[:,t-tile]^T @ v^T, one [128,1024] tile per t
            # row-max (over lv) -> mq; running elementwise max over t -> mv_acc
            mv_acc = pio.tile([128, L], BF16, tag="mv")
            mcols = psb.tile([128, 2, LT], F32, tag="mcols")  # [:,0,t]=m_q, [:,1,c]=m_v
            for t in range(LT):
                ps_s = pbig.tile([128, L], F32, tag="big")
                for kc in range(DC):
                    for n in range(2):
                        nc.tensor.matmul(
                            ps_s[:, ts(n, 512)],
                            lhsT=qwT[:, kc, ts(t, 128)],
                            rhs=vT[:, kc, ts(n, 512)],
                            start=(kc == 0),
                            stop=(kc == DC - 1),
                        )
                s_sb = psb.tile([128, L], BF16, tag="s")
                nc.scalar.copy(out=s_sb, in_=ps_s)
                nc.vector.reduce_max(out=mcols[:, 0, t : t + 1], in_=s_sb, axis=AX)
                if t == 0:
                    nc.vector.tensor_copy(out=mv_acc, in_=s_sb)
                else:
                    nc.vector.tensor_max(out=mv_acc, in0=mv_acc, in1=s_sb)

            # --- finalize m_v: transpose mv_acc 128-chunks, reduce over old partitions
            for g in range(LT // 2):
                ps_tr = pbig.tile([128, 256], BF16, tag="big")
                for j in range(2):
                    c = 2 * g + j
                    nc.tensor.transpose(ps_tr[:, ts(j, 128)], mv_acc[:, ts(c, 128)], ident)
                nc.vector.reduce_max(
                    out=mcols[:, 1, 2 * g : 2 * g + 2],
                    in_=ps_tr.rearrange("p (j x) -> p j x", j=2),
                    axis=AX,
                )

            # --- u = exp(tanh(m)) in [1/e, e]; no max-shift needed
            u_all = psb.tile([128, 2, LT], BF16, tag="uall")
            nc.scalar.activation(out=mcols, in_=mcols, func=mybir.ActivationFunctionType.Tanh)
            nc.scalar.activation(out=u_all, in_=mcols, func=mybir.ActivationFunctionType.Exp)
            den_vec = psb.tile([128, 2], F32, tag="denv")
            nc.vector.reduce_sum(out=den_vec, in_=u_all, axis=AX)

            # --- numerators sum_l u[l] * x[l,:] and denominators sum_l u[l]
            for sel, nat in ((0, q_nat), (1, v_nat)):
                acc = pacc.tile([1, D + 1], F32, tag="acc")
                for t in range(LT):
                    nc.tensor.matmul(
                        acc[0:1, 0:D],
                        lhsT=u_all[:, sel, t : t + 1],
                        rhs=nat[:, t, :],
                        start=(t == 0),
                        stop=(t == LT - 1),
                    )
                nc.tensor.matmul(
                    acc[0:1, D : D + 1],
                    lhsT=ones_col,
                    rhs=den_vec[:, sel : sel + 1],
                    start=True,
                    stop=True,
                )
                rden = patt.tile([1, 1], F32, tag="rden")
                nc.vector.reciprocal(out=rden, in_=acc[0:1, D : D + 1])
                att_row = patt.tile([1, D], F32, tag="att")
                nc.vector.tensor_scalar_mul(att_row, acc[0:1, 0:D], rden)
                nc.sync.dma_start(out=o_d[sel, b, :], in_=att_row)

    nc.compile()
    return nc


_NC_CACHE = None


def _get_nc():
    global _NC_CACHE
    if _NC_CACHE is None:
        _NC_CACHE = _build()
    return _NC_CACHE


def kernel(q, v, w):
    nc = _get_nc()
    q = np.asarray(q).astype(ml_dtypes.bfloat16)
    v = np.asarray(v).astype(ml_dtypes.bfloat16)
    w = np.asarray(w).astype(ml_dtypes.bfloat16)
    in_maps = [
        {
            "q": q[c * BPC : (c + 1) * BPC],
            "v": v[c * BPC : (c + 1) * BPC],
            "w": w,
        }
        for c in range(NCORES)
    ]
    res = run_bass_kernel_spmd(nc, in_maps, core_ids=list(range(NCORES)))
    outs = [res.results[c]["out"] for c in range(NCORES)]
    att_q = np.concatenate([o[0] for o in outs], axis=0)
    att_v = np.concatenate([o[1] for o in outs], axis=0)
    return att_q, att_v
